# revision 4
# baseline (speedup 1.0000x reference)
"""AttentionGRUCell fused kernel for 8 Trainium2 NeuronCores.

Data-parallel over the batch dim: each of the 8 cores processes a
2048-row shard of the 16384-row batch; the small weight matrices are
replicated.  Per core the cell is computed in four phases, one gate's
weight set (bf16, [128, 32, 1024] = 64 KiB/partition) resident at a
time:

  phase z: load x,h,a fp32 per 128-row tile -> cast bf16 -> one
    DMA-XBAR transpose into k-major [128, 32, 128] -> 64 matmuls into
    PSUM -> sigmoid -> z (fp16).  The transposed activations and z are
    spilled to DRAM.
  phase r: read back xhaT, 64 matmuls, sigmoid, rh = r*h (bf16),
    transpose rh, spill rhT.
  phase s: lhsT = [xT | rhT | aT], 64 matmuls -> tanh -> combine
    s = h + z*(tanh - h) in fp32 -> store s; transpose s -> spill sT.
  phase t: lhsT = [xT | aT | sT], 64 matmuls -> relu -> store t.

All matmuls use bf16 operands (stationary = transposed activation
tile, moving = 512-wide slice of the resident weights) with fp32 PSUM
accumulation.
"""

import sys

if "/opt/trn_rl_repo" not in sys.path:
    sys.path.insert(0, "/opt/trn_rl_repo")

import numpy as np

BATCH = 16384
EMB = 1024
HID = 1024
COMB = 2048
N_CORES = 8
B_L = BATCH // N_CORES          # rows per core
P = 128                         # partitions
N_BT_FULL = B_L // P            # batch tiles per core


def _build_nc(n_bt, with_bias):
    """Build + compile the per-core Bass program for n_bt batch tiles."""
    import concourse.mybir as mybir
    from contextlib import ExitStack
    from concourse import bacc
    from concourse.tile import TileContext

    dt = mybir.dt
    AF = mybir.ActivationFunctionType
    b_l = n_bt * P

    nc = bacc.Bacc("TRN2", target_bir_lowering=False, debug=False,
                   num_devices=N_CORES)

    x_d = nc.declare_dram_parameter("x", [b_l, EMB], dt.float32, isOutput=False)
    h_d = nc.declare_dram_parameter("h", [b_l, HID], dt.float32, isOutput=False)
    a_d = nc.declare_dram_parameter("a", [b_l, COMB], dt.float32, isOutput=False)
    wnames = ["Wz", "Uz", "Cz", "Wr", "Ur", "Cr",
              "W", "U", "C", "Vo", "Co", "Uo"]
    wshapes = {n: ([COMB, HID] if n.startswith("C") else [EMB, HID])
               for n in wnames}
    wd = {n: nc.declare_dram_parameter(n, wshapes[n], dt.float32,
                                       isOutput=False)
          for n in wnames}
    bias_d = {}
    if with_bias:
        for g in ("z", "r", "s", "t"):
            bias_d[g] = nc.declare_dram_parameter(
                f"bias_{g}", [P, HID], dt.float32, isOutput=False)
    s_out = nc.declare_dram_parameter("s", [b_l, HID], dt.float32, isOutput=True)
    t_out = nc.declare_dram_parameter("t", [b_l, HID], dt.float32, isOutput=True)

    # DRAM spill scratch, laid out to match the SBUF access order.
    xhaT_d = nc.dram_tensor("xhaT_spill", [n_bt, P, 32, P], dt.bfloat16)
    rhT_d = nc.dram_tensor("rhT_spill", [n_bt, P, 8, P], dt.bfloat16)
    sT_d = nc.dram_tensor("sT_spill", [n_bt, P, 8, P], dt.bfloat16)
    z_d = nc.dram_tensor("z_spill", [n_bt, P, HID], dt.float16)

    def load_weights_bf16(pool, staging, gate_blocks, tag):
        """gate_blocks: list of (dram_weight, n_kblocks). Returns a
        [128, 32, HID] bf16 resident tile."""
        total_k = sum(nkb for _, nkb in gate_blocks)
        wt = pool.tile([P, total_k, HID], dt.bfloat16, tag=tag)
        kb0 = 0
        i = 0
        for wdram, nkb in gate_blocks:
            for kb in range(nkb):
                stg = staging.tile([P, HID], dt.float32, tag="wstg")
                nc.sync.dma_start(stg[:], wdram[kb * P:(kb + 1) * P, :])
                if i % 2 == 0:
                    nc.vector.tensor_copy(wt[:, kb0 + kb, :], stg[:])
                else:
                    nc.scalar.activation(wt[:, kb0 + kb, :], stg[:], AF.Copy)
                i += 1
            kb0 += nkb
        return wt

    def mm_kloop(psums, lhs_of_kb, w, n_kb=32, start_kb=0, open_=True,
                 close=True):
        for kb in range(start_kb, start_kb + n_kb):
            st = open_ and kb == start_kb
            sp = close and kb == start_kb + n_kb - 1
            lhsT = lhs_of_kb(kb)
            nc.tensor.matmul(psums[0][:], lhsT, w[:, kb, 0:512],
                             start=st, stop=sp)
            nc.tensor.matmul(psums[1][:], lhsT, w[:, kb, 512:1024],
                             start=st, stop=sp)

    def evict(act_fn, psums, out_tile, bias_tile, ev, tagp):
        """out = act_fn(psum (+ bias)) over both 512-halves."""
        for half in range(2):
            src = psums[half]
            sl = slice(half * 512, half * 512 + 512)
            if bias_tile is not None:
                tmp = ev.tile([P, 512], dt.float32, tag=f"{tagp}{half}")
                nc.vector.tensor_add(tmp[:], src[:], bias_tile[:, sl])
                src = tmp
            nc.scalar.activation(out_tile[:, sl], src[:], act_fn)

    with TileContext(nc) as tc:
        # ------------------------------------------------------- phase z
        with ExitStack() as ph:
            wp = ph.enter_context(tc.tile_pool(name="wz", bufs=1))
            stg = ph.enter_context(tc.tile_pool(name="stgz", bufs=2))
            w = load_weights_bf16(wp, stg,
                                  [(wd["Wz"], 8), (wd["Uz"], 8), (wd["Cz"], 16)],
                                  "w")
            bz = None
            if with_bias:
                bz = wp.tile([P, HID], dt.float32, tag="bias")
                nc.sync.dma_start(bz[:], bias_d["z"][:])
            ld = ph.enter_context(tc.tile_pool(name="ldz", bufs=2))
            bf = ph.enter_context(tc.tile_pool(name="bfz", bufs=2))
            tp = ph.enter_context(tc.tile_pool(name="tpz", bufs=2))
            ev = ph.enter_context(tc.tile_pool(name="evz", bufs=2))
            ps = ph.enter_context(tc.tile_pool(name="psz", bufs=3, space="PSUM"))

            for bt in range(n_bt):
                r0 = bt * P
                xha = ld.tile([P, 4096], dt.float32, tag="xha")
                nc.sync.dma_start(xha[:, 0:1024], x_d[r0:r0 + P, :])
                nc.sync.dma_start(xha[:, 1024:2048], h_d[r0:r0 + P, :])
                nc.sync.dma_start(xha[:, 2048:4096], a_d[r0:r0 + P, :])
                xha_b = bf.tile([P, 4096], dt.bfloat16, tag="xha_b")
                nc.vector.tensor_copy(xha_b[:, 0:2048], xha[:, 0:2048])
                nc.scalar.activation(xha_b[:, 2048:4096], xha[:, 2048:4096],
                                     AF.Copy)
                xhaT = tp.tile([P, 32, P], dt.bfloat16, tag="xhaT")
                nc.scalar.dma_start(xhaT[:], xha_b[:], transpose=True)

                p0 = ps.tile([P, 512], dt.float32, tag="p0")
                p1 = ps.tile([P, 512], dt.float32, tag="p1")
                mm_kloop((p0, p1), lambda kb: xhaT[:, kb, :], w)

                z16 = ev.tile([P, HID], dt.float16, tag="z16")
                evict(AF.Sigmoid, (p0, p1), z16, bz, ev, "zb")
                nc.sync.dma_start(xhaT_d[bt], xhaT[:])
                nc.sync.dma_start(z_d[bt], z16[:])

        # ------------------------------------------------------- phase r
        with ExitStack() as ph:
            wp = ph.enter_context(tc.tile_pool(name="wr", bufs=1))
            stg = ph.enter_context(tc.tile_pool(name="stgr", bufs=2))
            w = load_weights_bf16(wp, stg,
                                  [(wd["Wr"], 8), (wd["Ur"], 8), (wd["Cr"], 16)],
                                  "w")
            br = None
            if with_bias:
                br = wp.tile([P, HID], dt.float32, tag="bias")
                nc.sync.dma_start(br[:], bias_d["r"][:])
            ld = ph.enter_context(tc.tile_pool(name="ldr", bufs=2))
            tp = ph.enter_context(tc.tile_pool(name="tpr", bufs=2))
            ev = ph.enter_context(tc.tile_pool(name="evr", bufs=2))
            ps = ph.enter_context(tc.tile_pool(name="psr", bufs=3, space="PSUM"))

            for bt in range(n_bt):
                r0 = bt * P
                xhaT = tp.tile([P, 32, P], dt.bfloat16, tag="xhaT")
                nc.sync.dma_start(xhaT[:], xhaT_d[bt])
                hf = ld.tile([P, HID], dt.float32, tag="hf")
                nc.sync.dma_start(hf[:], h_d[r0:r0 + P, :])

                p0 = ps.tile([P, 512], dt.float32, tag="p0")
                p1 = ps.tile([P, 512], dt.float32, tag="p1")
                mm_kloop((p0, p1), lambda kb: xhaT[:, kb, :], w)

                rf = ev.tile([P, HID], dt.float32, tag="rf")
                evict(AF.Sigmoid, (p0, p1), rf, br, ev, "rb")
                rhb = ev.tile([P, HID], dt.bfloat16, tag="rhb")
                nc.vector.tensor_mul(rhb[:], rf[:], hf[:])
                rhT = tp.tile([P, 8, P], dt.bfloat16, tag="rhT")
                nc.scalar.dma_start(rhT[:], rhb[:], transpose=True)
                nc.sync.dma_start(rhT_d[bt], rhT[:])

        # ------------------------------------------------------- phase s
        with ExitStack() as ph:
            wp = ph.enter_context(tc.tile_pool(name="wsp", bufs=1))
            stg = ph.enter_context(tc.tile_pool(name="stgs", bufs=2))
            w = load_weights_bf16(wp, stg,
                                  [(wd["W"], 8), (wd["U"], 8), (wd["C"], 16)],
                                  "w")
            bs = None
            if with_bias:
                bs = wp.tile([P, HID], dt.float32, tag="bias")
                nc.sync.dma_start(bs[:], bias_d["s"][:])
            ld = ph.enter_context(tc.tile_pool(name="lds", bufs=2))
            tp = ph.enter_context(tc.tile_pool(name="tps", bufs=2))
            ev = ph.enter_context(tc.tile_pool(name="evs", bufs=2))
            ps = ph.enter_context(tc.tile_pool(name="pss", bufs=3, space="PSUM"))

            for bt in range(n_bt):
                r0 = bt * P
                lT = tp.tile([P, 32, P], dt.bfloat16, tag="lT")
                nc.sync.dma_start(lT[:, 0:8, :], xhaT_d[bt][:, 0:8, :])
                nc.sync.dma_start(lT[:, 8:16, :], rhT_d[bt])
                nc.sync.dma_start(lT[:, 16:32, :], xhaT_d[bt][:, 16:32, :])
                z16 = ld.tile([P, HID], dt.float16, tag="z16")
                nc.sync.dma_start(z16[:], z_d[bt])
                hf = ld.tile([P, HID], dt.float32, tag="hf")
                nc.sync.dma_start(hf[:], h_d[r0:r0 + P, :])

                p0 = ps.tile([P, 512], dt.float32, tag="p0")
                p1 = ps.tile([P, 512], dt.float32, tag="p1")
                mm_kloop((p0, p1), lambda kb: lT[:, kb, :], w)

                stil = ev.tile([P, HID], dt.float32, tag="stil")
                evict(AF.Tanh, (p0, p1), stil, bs, ev, "sb")
                dlt = ev.tile([P, HID], dt.float32, tag="dlt")
                zd = ev.tile([P, HID], dt.float32, tag="zd")
                sf = ev.tile([P, HID], dt.float32, tag="sf")
                nc.vector.tensor_sub(dlt[:], stil[:], hf[:])
                nc.vector.tensor_mul(zd[:], z16[:], dlt[:])
                nc.vector.tensor_add(sf[:], hf[:], zd[:])
                nc.sync.dma_start(s_out[r0:r0 + P, :], sf[:])

                sb2 = ev.tile([P, HID], dt.bfloat16, tag="sb16")
                nc.vector.tensor_copy(sb2[:], sf[:])
                sT = tp.tile([P, 8, P], dt.bfloat16, tag="sT")
                nc.scalar.dma_start(sT[:], sb2[:], transpose=True)
                nc.sync.dma_start(sT_d[bt], sT[:])

        # ------------------------------------------------------- phase t
        with ExitStack() as ph:
            wp = ph.enter_context(tc.tile_pool(name="wtp", bufs=1))
            stg = ph.enter_context(tc.tile_pool(name="stgt", bufs=2))
            w = load_weights_bf16(wp, stg,
                                  [(wd["Vo"], 8), (wd["Co"], 16), (wd["Uo"], 8)],
                                  "w")
            btl = None
            if with_bias:
                btl = wp.tile([P, HID], dt.float32, tag="bias")
                nc.sync.dma_start(btl[:], bias_d["t"][:])
            tp = ph.enter_context(tc.tile_pool(name="tpt", bufs=2))
            ev = ph.enter_context(tc.tile_pool(name="evt", bufs=2))
            ps = ph.enter_context(tc.tile_pool(name="pst", bufs=3, space="PSUM"))

            for bt in range(n_bt):
                r0 = bt * P
                lT = tp.tile([P, 32, P], dt.bfloat16, tag="lT")
                nc.sync.dma_start(lT[:, 0:8, :], xhaT_d[bt][:, 0:8, :])
                nc.sync.dma_start(lT[:, 8:24, :], xhaT_d[bt][:, 16:32, :])
                nc.sync.dma_start(lT[:, 24:32, :], sT_d[bt])

                p0 = ps.tile([P, 512], dt.float32, tag="p0")
                p1 = ps.tile([P, 512], dt.float32, tag="p1")
                mm_kloop((p0, p1), lambda kb: lT[:, kb, :], w)

                tf = ev.tile([P, HID], dt.float32, tag="tf")
                evict(AF.Relu, (p0, p1), tf, btl, ev, "tb")
                nc.sync.dma_start(t_out[r0:r0 + P, :], tf[:])

    nc.compile()
    return nc


_CACHE = {}


def _get_exec(n_bt, with_bias):
    """Build (once per process) the compiled program and a sharded jit
    callable over the 8 cores, mirroring bass2jax.run_bass_via_pjrt."""
    key = (n_bt, with_bias)
    if key in _CACHE:
        return _CACHE[key]

    import jax
    import concourse.mybir as mybir
    from concourse import bass2jax
    from jax.sharding import Mesh, PartitionSpec
    from jax.experimental.shard_map import shard_map

    bass2jax.install_neuronx_cc_hook()
    nc = _build_nc(n_bt, with_bias)

    partition_name = (nc.partition_id_tensor.name
                      if nc.partition_id_tensor else None)
    in_names = []
    out_names = []
    out_avals = []
    zero_outs = []
    for alloc in nc.m.functions[0].allocations:
        if not isinstance(alloc, mybir.MemoryLocationSet):
            continue
        name = alloc.memorylocations[0].name
        if alloc.kind == "ExternalInput":
            if name != partition_name:
                in_names.append(name)
        elif alloc.kind == "ExternalOutput":
            out_names.append(name)
            shape = tuple(alloc.tensor_shape)
            dtype = mybir.dt.np(alloc.dtype)
            out_avals.append(jax.core.ShapedArray(shape, dtype))
            zero_outs.append(np.zeros(shape, dtype))
    n_params = len(in_names)
    all_in_names = in_names + out_names
    if partition_name is not None:
        all_in_names = all_in_names + [partition_name]

    def _body(*args):
        operands = list(args)
        if partition_name is not None:
            operands.append(bass2jax.partition_id_tensor())
        outs = bass2jax._bass_exec_p.bind(
            *operands,
            out_avals=tuple(out_avals),
            in_names=tuple(all_in_names),
            out_names=tuple(out_names),
            lowering_input_output_aliases=(),
            sim_require_finite=True,
            sim_require_nnan=True,
            nc=nc,
        )
        return tuple(outs)

    devices = jax.devices()[:N_CORES]
    mesh = Mesh(np.asarray(devices), ("core",))
    n_outs = len(out_names)
    sharded = jax.jit(
        shard_map(
            _body, mesh=mesh,
            in_specs=(PartitionSpec("core"),) * (n_params + n_outs),
            out_specs=(PartitionSpec("core"),) * n_outs,
            check_rep=False,
        ),
        keep_unused=True,
    )
    entry = {
        "nc": nc,
        "sharded": sharded,
        "in_names": in_names,
        "out_names": out_names,
        "zero_outs": zero_outs,
        "mesh": mesh,
    }
    _CACHE[key] = entry
    return entry


def _prepare_in_arrays(entry, inputs, bias_rows):
    """Concatenated (8*shape[0], ...) global arrays in BIR input order."""
    per_core = {
        "x": inputs["in_word"],
        "h": inputs["last_hid_state"],
        "a": inputs["attended_state"],
    }
    arrs = []
    for name in entry["in_names"]:
        if name in per_core:
            arrs.append(np.ascontiguousarray(per_core[name], dtype=np.float32))
        elif name.startswith("bias_"):
            g = name.split("_")[1]
            row = np.broadcast_to(np.asarray(bias_rows[g], np.float32), (P, HID))
            arrs.append(np.ascontiguousarray(np.tile(row, (N_CORES, 1))))
        else:
            w = np.asarray(inputs[name], dtype=np.float32)
            arrs.append(np.ascontiguousarray(np.tile(w, (N_CORES, 1))))
    return arrs


def kernel(in_word, last_hid_state, attended_state,
           W, bw, Wz, bwz, Wr, bwr,
           U, bu, Uz, buz, Ur, bur,
           C, bc, Cz, bcz, Cr, bcr,
           Uo, buo, Vo, bvo, Co, bco):
    inputs = dict(in_word=np.asarray(in_word),
                  last_hid_state=np.asarray(last_hid_state),
                  attended_state=np.asarray(attended_state),
                  W=W, Wz=Wz, Wr=Wr, U=U, Uz=Uz, Ur=Ur,
                  C=C, Cz=Cz, Cr=Cr, Uo=Uo, Vo=Vo, Co=Co)
    bias_rows = {
        "z": np.asarray(bwz) + np.asarray(buz) + np.asarray(bcz),
        "r": np.asarray(bwr) + np.asarray(bur) + np.asarray(bcr),
        "s": np.asarray(bw) + np.asarray(bu) + np.asarray(bc),
        "t": np.asarray(buo) + np.asarray(bvo) + np.asarray(bco),
    }
    with_bias = any(np.any(v != 0) for v in bias_rows.values())

    entry = _get_exec(N_BT_FULL, with_bias)
    arrs = _prepare_in_arrays(entry, inputs, bias_rows)
    zeros = [np.zeros((N_CORES * z.shape[0], *z.shape[1:]), z.dtype)
             for z in entry["zero_outs"]]
    outs = entry["sharded"](*arrs, *zeros)
    res = {name: np.asarray(outs[i]) for i, name in enumerate(entry["out_names"])}
    return (res["s"], res["t"])
